# revision 64
# baseline (speedup 1.0000x reference)
"""Phi-2-style attention layer (B=1, L=2048, D=2560, 32 heads, partial rope 32)
as a distributed Bass kernel on 8 TRN2 NeuronCores.

Sharding: tensor-parallel over heads (4 heads/core).
  - x is replicated, passed as contiguous 256-column chunks xTc [8, 2560, 256].
  - Each core computes rope'd qT/kT + v for its 4 heads, causal attention in
    the S^T (k-on-partitions) layout, then the normalized attention output
    attnT [320, 2048] is AllGathered to [2560, 2048]; each core computes its
    320-column slice of the output projection. Host concatenates the slices.

Attention phase (CD) is software-pipelined for PE occupancy:
  - per (qc, head-pair): S^T for both heads goes into one [128, 2, 512] PSUM
    mega-tile (2 banks); one batched exp per kb covers both heads.
  - score/PV matmuls are trimmed to live columns (true-causal): for a
    diagonal block only cols >= c0 are computed, and the additive mask is
    applied on a 128-wide strip only (DVE) with a separate small exp.
  - exp output (est) and V are bf16; denominator comes from a ones-column at
    padded V column 96.
  - out-proj for round qc is deferred two rounds so its AllGather hides
    behind later rounds' attention matmuls.

All matmuls run in bf16 (f32r for tiny bias rank-1), fp32 PSUM accumulation.
Softmax is computed unnormalized (scores are O(5); exp without
max-subtraction is safe; mask -1e9 underflows exp to exactly 0).
"""

import math
from contextlib import ExitStack

import numpy as np

L = 2048
C = 2560
NCORES = 8
HPC = 4          # heads per core
HD = 80          # head dim
DH = HPC * HD    # 320 dims per core
ROT = 32
RD = ROT // 2    # 16
BASE = 10000.0
KT = C // 128    # 20 k tiles
NW = 256         # x chunk width
NCH = L // NW    # 8 chunks
SC = 1.0 / math.sqrt(HD)

_RUNNERS = {}


def _build(mask_plan, reps=1, sim_mode=False):
    import concourse.bacc as bacc
    import concourse.tile as tile
    from concourse import mybir

    f32 = mybir.dt.float32
    f32r = mybir.dt.float32r
    entries, nmix, maxw = mask_plan

    nc = bacc.Bacc("TRN2", target_bir_lowering=False, debug=False,
                   num_devices=NCORES)

    d = {}
    bf16_ = mybir.dt.bfloat16
    d["xTc"] = nc.dram_tensor("xTc", [NCH, C, NW], bf16_, kind="ExternalInput").ap()
    d["wqk"] = nc.dram_tensor("wqk_t", [C, 2 * DH], bf16_, kind="ExternalInput").ap()
    d["wv"] = nc.dram_tensor("wv_t", [C, DH], bf16_, kind="ExternalInput").ap()
    d["wo"] = nc.dram_tensor("wo_t", [C, DH], bf16_, kind="ExternalInput").ap()
    d["bqk"] = nc.dram_tensor("bqk", [128, 5], f32, kind="ExternalInput").ap()
    d["cb"] = nc.dram_tensor("cb", [1, DH], f32r, kind="ExternalInput").ap()
    d["maskm"] = nc.dram_tensor("maskm", [max(nmix, 1), 128, max(maxw, 1)],
                                bf16_, kind="ExternalInput").ap()
    d["cos"] = nc.dram_tensor("cos8", [128, L], bf16_, kind="ExternalInput").ap()
    d["sin"] = nc.dram_tensor("sin8", [128, L], bf16_, kind="ExternalInput").ap()
    d["ones"] = nc.dram_tensor("ones128", [1, 128], f32r, kind="ExternalInput").ap()
    d["vpad"] = nc.dram_tensor("vpad", [128, 16, HPC, 48], bf16_,
                               kind="ExternalInput").ap()
    d["y"] = nc.dram_tensor("y", [L, DH], f32, kind="ExternalOutput").ap()

    with tile.TileContext(nc) as tc:
        with ExitStack() as gctx:
            P = {
                "small": gctx.enter_context(
                    tc.tile_pool(name="smallg", bufs=2)),
                "wo": gctx.enter_context(tc.tile_pool(name="wog", bufs=2)),
                "g": gctx.enter_context(tc.tile_pool(name="gg", bufs=2)),
                "dram": gctx.enter_context(
                    tc.tile_pool(name="dramg", bufs=1, space="DRAM")),
            }
            carry = []
            for rep in range(reps):
                carry = _build_rep(nc, tc, mybir, f32, f32r, d, mask_plan,
                                   rep, sim_mode, P, carry,
                                   is_last=(rep == reps - 1))
    nc.compile()
    return nc


def _rest_runs(mi):
    """For qk m-tile mi in {2,3,4}: contiguous runs (is_q, head, dst_row,
    src_row, nrows) mapping eviction rows to per-head tiles."""
    runs = []
    p = 0
    while p < 128:
        r = (mi - 2) * 128 + p
        if r < 192:
            h, j, is_q = r // 48, r % 48, True
        else:
            h, j, is_q = (r - 192) // 48, (r - 192) % 48, False
        n = min(48 - j, 128 - p)
        runs.append((is_q, h, 32 + j, p, n))
        p += n
    return runs


def _outproj_pieces(nc, P, d, qc, ag_q, wo_sb, ones_sb, cb_sb, mybir,
                    gq="sync"):
    """Out-projection for one 512-wide q round, split into small pieces
    (one g-load + 8 matmul chunks) that callers interleave into attention
    kb-loops as PE filler. PSUM/eviction tiles come from the pools of the
    rep that RUNS the piece (P["psD_cur"]/P["ye_cur"]), so pieces may be
    carried across rep boundaries; g/wo/ones/cb data tiles are captured
    from the building rep (persistent pools, bufs=2)."""
    f32 = mybir.dt.float32
    bf16 = mybir.dt.bfloat16
    g = P["g"].tile([128, KT, 512], bf16, name="g", tag="g")

    def gload():
        eng = nc.gpsimd if gq == "gpsimd" else nc.sync
        ag_r = ag_q[qc].rearrange("(t p) w -> p t w", p=128)
        eng.dma_start(out=g[:, 0:10, :], in_=ag_r[:, 0:10, :])
        eng.dma_start(out=g[:, 10:KT, :], in_=ag_r[:, 10:KT, :])

    pieces = [("g", gload)]
    psys = {}
    for jj in range(4):
        lb = qc * 4 + jj
        for kc in range(4):
            def pk(jj=jj, lb=lb, kc=kc):
                if kc == 0:
                    psy = P["psD_cur"].tile([128, DH], f32, name="psy",
                                            tag="psy")
                    psys[jj] = psy
                else:
                    psy = psys[jj]
                for k in range(5 * kc, 5 * kc + 5):
                    nc.tensor.matmul(psy[:],
                                     g[:, k, jj * 128:(jj + 1) * 128],
                                     wo_sb[:, k, :],
                                     start=(k == 0), stop=False)
                if kc == 3:
                    psys.pop(jj)
                    nc.tensor.matmul(psy[:], ones_sb[:], cb_sb[:],
                                     start=False, stop=True)
                    ye = P["ye_cur"].tile([128, DH], f32, name="ye",
                                          tag="ye")
                    nc.vector.tensor_scalar_add(ye[:], psy[:], 0.0)
                    nc.gpsimd.dma_start(
                        out=d["y"][lb * 128:(lb + 1) * 128, :], in_=ye[:])

            pieces.append(("mm", pk))
    return pieces


def _build_rep(nc, tc, mybir, f32, f32r, d, mask_plan, rep, sim_mode, P,
               carry, is_last):
    Exp = mybir.ActivationFunctionType.Exp
    Ln = mybir.ActivationFunctionType.Ln
    Identity = mybir.ActivationFunctionType.Identity
    Copy = mybir.ActivationFunctionType.Copy
    bypass = mybir.AluOpType.bypass
    add_op = mybir.AluOpType.add
    bf16 = mybir.dt.bfloat16
    entries, nmix, maxw = mask_plan

    wqk_r = d["wqk"].rearrange("(t p) m -> p t m", p=128)
    wv_r = d["wv"].rearrange("(t p) m -> p t m", p=128)
    wo_r = d["wo"].rearrange("(t p) m -> p t m", p=128)

    live_kb = {qc: [kb for kb in range(16) if entries[kb][qc] is not None]
               for qc in range(4)}

    with ExitStack() as ctx:
        small = P["small"]
        ones_sb = small.tile([1, 128], f32r, name="ones", tag="ones")
        nc.scalar.dma_start(out=ones_sb[:], in_=d["ones"][:])
        cb_sb = small.tile([1, DH], f32r, name="cb", tag="cb")
        nc.scalar.dma_start(out=cb_sb[:], in_=d["cb"][:])
        bqk_sb = small.tile([128, 5], f32, name="bqk", tag="bqk")
        nc.scalar.dma_start(out=bqk_sb[:], in_=d["bqk"][:])
        dpool = P["dram"]
        bounce_q = [dpool.tile([DH, 512], bf16, name=f"bnc{qc}",
                               tag=f"bnc{qc}") for qc in range(4)]
        ag_q = [dpool.tile([C, 512], bf16, name=f"agq{qc}", tag=f"agq{qc}",
                           addr_space="Local" if sim_mode else "Shared")
                for qc in range(4)]

        with ExitStack() as qctx:  # phases AB + CD
            qkv_pool = qctx.enter_context(
                tc.tile_pool(name=f"qkv{rep}", bufs=1))
            qh = [qkv_pool.tile([128, L], bf16, name=f"qh{h}", tag=f"qh{h}")
                  for h in range(HPC)]
            kh = [qkv_pool.tile([128, L], bf16, name=f"kh{h}", tag=f"kh{h}")
                  for h in range(HPC)]
            v_sb = qkv_pool.tile([128, 16, HPC, 128], bf16, name="v", tag="v")
            nc.gpsimd.dma_start(out=v_sb[:, :, :, HD:128], in_=d["vpad"][:])

            # ---------- Phase AB: full QKV projection + rope + scatter ------
            with ExitStack() as actx:
                cpool = actx.enter_context(
                    tc.tile_pool(name=f"cs{rep}", bufs=1))
                cos_sb = cpool.tile([128, L], bf16, name="cos", tag="cos")
                sin_sb = cpool.tile([128, L], bf16, name="sin", tag="sin")
                nc.scalar.dma_start(out=cos_sb[:], in_=d["cos"][:])
                nc.scalar.dma_start(out=sin_sb[:], in_=d["sin"][:])
                s1f = cpool.tile([128, L], bf16, name="s1f", tag="s1f")
                s2f = cpool.tile([128, L], bf16, name="s2f", tag="s2f")
                wpool = actx.enter_context(
                    tc.tile_pool(name=f"wab{rep}", bufs=1))
                xpool = actx.enter_context(
                    tc.tile_pool(name=f"xab{rep}", bufs=2))
                epool = actx.enter_context(
                    tc.tile_pool(name=f"evab{rep}", bufs=2))
                psA = actx.enter_context(
                    tc.tile_pool(name=f"psab{rep}", bufs=1, space="PSUM"))

                wqk_sb = wpool.tile([128, KT, 2 * DH], bf16, name="wqk",
                                    tag="wqk")
                wv_sb = wpool.tile([128, KT, DH], bf16, name="wv", tag="wv")
                # split by k-half so khf=0 matmuls start after half the
                # bytes; on sync: the sync engine is idle at the end of the
                # previous rep's CD, so these run during it (the scalar
                # engine is stuck behind that rep's exp stream)
                nc.sync.dma_start(out=wqk_sb[:, 0:10, :],
                                  in_=wqk_r[:, 0:10, :])
                nc.sync.dma_start(out=wv_sb[:, 0:10, :],
                                  in_=wv_r[:, 0:10, :])
                nc.sync.dma_start(out=wqk_sb[:, 10:20, :],
                                  in_=wqk_r[:, 10:20, :])
                nc.sync.dma_start(out=wv_sb[:, 10:20, :],
                                  in_=wv_r[:, 10:20, :])

                for n in range(NCH):
                    nsl = slice(n * NW, (n + 1) * NW)
                    psm = [psA.tile([128, NW], f32, name=f"psm{m}",
                                    tag=f"psm{m}") for m in range(5)]
                    psv = [psA.tile([128, DH], f32, name=f"psv{j}",
                                    tag=f"psv{j}") for j in range(2)]
                    for khf in range(2):
                        xa = xpool.tile([128, 10, NW], bf16, name="xa",
                                        tag="xa")
                        nc.sync.dma_start(
                            out=xa[:],
                            in_=d["xTc"][n, khf * 1280:(khf + 1) * 1280, :]
                            .rearrange("(t p) w -> p t w", p=128))
                        for m in range(5):
                            for kk in range(10):
                                k = khf * 10 + kk
                                nc.tensor.matmul(
                                    psm[m][:],
                                    wqk_sb[:, k, m * 128:(m + 1) * 128],
                                    xa[:, kk, :],
                                    start=(k == 0), stop=(k == KT - 1))
                        for j in range(2):
                            for kk in range(10):
                                k = khf * 10 + kk
                                nc.tensor.matmul(
                                    psv[j][:],
                                    xa[:, kk, j * 128:(j + 1) * 128],
                                    wv_sb[:, k, :],
                                    start=(k == 0), stop=(k == KT - 1))
                    # evictions
                    ev = {}
                    for m in range(5):
                        e = epool.tile([128, NW], bf16, name=f"e{m}",
                                       tag=f"e{m}")
                        nc.scalar.activation(e[:], psm[m][:], Identity,
                                             bias=bqk_sb[:, m:m + 1])
                        ev[m] = e
                    for j in range(2):
                        nc.scalar.activation(
                            v_sb[:, 2 * n + j, :, 0:HD],
                            psv[j][:].rearrange("p (h dd) -> p h dd", h=HPC),
                            Copy)
                    # rope chunk
                    co, si = cos_sb[:, nsl], sin_sb[:, nsl]
                    rt1 = epool.tile([128, NW], f32, name="rt1", tag="rt1")
                    rt2 = epool.tile([128, NW], f32, name="rt2", tag="rt2")
                    nc.vector.tensor_mul(rt1[:], ev[0][:], co)
                    nc.vector.tensor_mul(rt2[:], ev[1][:], si)
                    nc.vector.tensor_sub(s1f[:, nsl], rt1[:], rt2[:])
                    nc.vector.tensor_mul(rt1[:], ev[0][:], si)
                    nc.vector.tensor_mul(rt2[:], ev[1][:], co)
                    nc.vector.tensor_add(s2f[:, nsl], rt1[:], rt2[:])
                    # scatter rest rows of m=2,3,4
                    for m in (2, 3, 4):
                        for is_q, h, dr, sr, nr in _rest_runs(m):
                            dst = qh[h] if is_q else kh[h]
                            nc.scalar.dma_start(
                                out=dst[dr:dr + nr, nsl],
                                in_=ev[m][sr:sr + nr, :])

                for h in range(HPC):
                    nc.gpsimd.dma_start(out=qh[h][0:16, :],
                                        in_=s1f[16 * h:16 * h + 16, :])
                    nc.sync.dma_start(out=qh[h][16:32, :],
                                      in_=s2f[16 * h:16 * h + 16, :])
                    nc.gpsimd.dma_start(out=kh[h][0:16, :],
                                        in_=s1f[64 + 16 * h:80 + 16 * h, :])
                    nc.sync.dma_start(out=kh[h][16:32, :],
                                      in_=s2f[64 + 16 * h:80 + 16 * h, :])

            # ---------- Phase CD: attention + per-qc AllGather + out-proj ---
            with ExitStack() as cctx:
                mpool = cctx.enter_context(
                    tc.tile_pool(name=f"maskp{rep}", bufs=1))
                epool = cctx.enter_context(
                    tc.tile_pool(name=f"est{rep}", bufs=3))
                smpool = cctx.enter_context(
                    tc.tile_pool(name=f"smp{rep}", bufs=2))
                atpool = cctx.enter_context(
                    tc.tile_pool(name=f"attnq{rep}", bufs=2))
                npool = cctx.enter_context(
                    tc.tile_pool(name=f"norm{rep}", bufs=2))
                ypool = cctx.enter_context(
                    tc.tile_pool(name=f"ye{rep}", bufs=2))
                ps_st = cctx.enter_context(
                    tc.tile_pool(name=f"psST{rep}", bufs=2, space="PSUM"))
                ps_pv = cctx.enter_context(
                    tc.tile_pool(name=f"psPV{rep}", bufs=1, space="PSUM"))
                psD = cctx.enter_context(
                    tc.tile_pool(name=f"psD{rep}", bufs=2, space="PSUM"))

                pvcpool = cctx.enter_context(
                    tc.tile_pool(name=f"pvc{rep}", bufs=2))
                P["psD_cur"] = psD
                P["ye_cur"] = ypool

                # compact mask strips, loaded once
                mq = None
                if nmix:
                    mq = mpool.tile([128, nmix, maxw], bf16, name="mq",
                                    tag="mq")
                    nc.sync.dma_start(out=mq[:], in_=d["maskm"]
                                      .rearrange("b p w -> p b w"))
                wo_sb = P["wo"].tile([128, KT, DH], bf16, name="wo",
                                     tag="wo")
                nc.sync.dma_start(out=wo_sb[:], in_=wo_r[:])

                # deferred normalize/bounce/collective work: emitted a few
                # kb-iterations into the NEXT segment so slow per-head chains
                # never head-of-line-block the attention engines
                pending = []

                def flush_pending():
                    for f in pending:
                        f()
                    pending.clear()

                def mk_norm(qc, hp, pvc, lnd):
                    def f():
                        for j in range(2):
                            h = 2 * hp + j
                            den = npool.tile([1, 512], f32, name="den",
                                             tag="den")
                            nc.scalar.activation(den[:], lnd[j][:],
                                                 Exp, scale=-1.0)
                            denb = npool.tile([HD, 512], f32, name="denb",
                                              tag="denb")
                            nc.gpsimd.partition_broadcast(denb[:], den[:])
                            attnq = atpool.tile([HD, 512], bf16,
                                                name="attnq", tag="attnq")
                            nc.vector.tensor_mul(attnq[:],
                                                 pvc[0:HD, j, :], denb[:])
                            nc.gpsimd.dma_start(
                                out=bounce_q[qc][h * HD:(h + 1) * HD, :],
                                in_=attnq[:])
                        if hp == 0:
                            return
                        if sim_mode:
                            for rr in range(NCORES):
                                nc.gpsimd.dma_start(
                                    out=ag_q[qc][rr * DH:(rr + 1) * DH, :],
                                    in_=bounce_q[qc][:])
                        else:
                            nc.gpsimd.collective_compute(
                                "AllGather",
                                bypass,
                                replica_groups=[list(range(NCORES))],
                                ins=[bounce_q[qc].opt()],
                                outs=[ag_q[qc].opt()],
                            )
                    return f

                # st tiles are emitted two work-items ahead of their exp/pv
                # consumers, ACROSS segment boundaries, so the PE never sits
                # behind a cold exp pipeline at a segment start
                sts = {}

                def emit_st(qc, hp, kb):
                    dd, lo, wid, sidx = entries[kb][qc]
                    st = ps_st.tile([128, 2, 512], f32, name="st", tag="st")
                    for j in range(2):
                        h = 2 * hp + j
                        nc.tensor.matmul(
                            st[:, j, dd:512],
                            kh[h][0:HD, kb * 128:(kb + 1) * 128],
                            qh[h][0:HD, qc * 512 + dd:qc * 512 + 512],
                            start=True, stop=True)
                    sts[(qc, hp, kb)] = st

                def attention_segment(qc, hp, projq, nxt=None,
                                      pop_from=3, stride=1):
                    kbs = live_kb[qc]
                    if projq and projq[0][0] == "g":
                        projq.pop(0)[1]()
                    pv = [ps_pv.tile([128, 512], f32, name=f"pv{j}",
                                     tag=f"pv{j}") for j in range(2)]
                    for kb in kbs[:2]:
                        if (qc, hp, kb) not in sts:
                            emit_st(qc, hp, kb)
                    for i, kb in enumerate(kbs):
                        if i + 2 < len(kbs):
                            emit_st(qc, hp, kbs[i + 2])
                        elif nxt is not None:
                            nqc, nhp = nxt
                            nkbs = live_kb[nqc]
                            k = i + 2 - len(kbs)
                            if k < min(2, len(nkbs)):
                                emit_st(nqc, nhp, nkbs[k])
                        # out-proj matmul pieces as PE filler while the
                        # scalar engine works through the exps
                        if (i >= pop_from and (i - pop_from) % stride == 0
                                and projq):
                            projq.pop(0)[1]()
                        if i == min(2, len(kbs) - 1):
                            flush_pending()
                        dd, lo, wid, sidx = entries[kb][qc]
                        st = sts.pop((qc, hp, kb))
                        est = epool.tile([128, 2, 512], bf16, name="est",
                                         tag="est")
                        if wid:
                            hi = lo + wid
                            sm = smpool.tile([128, 2, maxw], f32,
                                             name="sm", tag="sm")
                            for j in range(2):
                                nc.vector.scalar_tensor_tensor(
                                    out=sm[:, j, 0:wid],
                                    in0=st[:, j, lo:hi], scalar=1.0,
                                    in1=mq[:, sidx, 0:wid],
                                    op0=bypass, op1=add_op)
                            if lo > dd:
                                nc.scalar.activation(
                                    est[:, :, dd:lo], st[:, :, dd:lo], Exp)
                            if hi < 512:
                                nc.scalar.activation(
                                    est[:, :, hi:512], st[:, :, hi:512],
                                    Exp)
                            nc.scalar.activation(
                                est[:, :, lo:hi], sm[:, :, 0:wid], Exp)
                        else:
                            nc.scalar.activation(
                                est[:, :, dd:512], st[:, :, dd:512], Exp)
                        for j in range(2):
                            h = 2 * hp + j
                            nc.tensor.matmul(
                                pv[j][:, dd:512],
                                v_sb[:, kb, h, :],
                                est[:, j, dd:512],
                                start=(i == 0),
                                stop=(i == len(kbs) - 1))
                    # release PSUM fast: DVE evicts pv, Act takes ln of the
                    # denominator rows; the rest of the normalize chain is
                    # deferred via `pending`
                    pvc = pvcpool.tile([128, 2, 512], f32, name="pvc",
                                       tag="pvc")
                    lnd = [npool.tile([1, 512], f32, name=f"lnd{j}",
                                      tag=f"lnd{j}") for j in range(2)]
                    for j in range(2):
                        nc.vector.tensor_scalar_add(pvc[:, j, :],
                                                    pv[j][:], 0.0)
                        nc.scalar.activation(lnd[j][:],
                                             pv[j][96:97, :], Ln)
                    pending.append(mk_norm(qc, hp, pvc, lnd))

                order = (0, 1, 2, 3)
                projq = list(carry)
                for qi, qc in enumerate(order):
                    if qi >= 2:
                        projq += _outproj_pieces(
                            nc, P, d, order[qi - 2], ag_q, wo_sb, ones_sb,
                            cb_sb, mybir)
                    if qi == 3:
                        # proj of order[2] also rides inside round order[3]
                        # (second head-pair) instead of trailing the rep;
                        # its g-load goes via gpsimd so the sync queue is
                        # free for the next rep's weight/x prefetches
                        projq += _outproj_pieces(
                            nc, P, d, order[2], ag_q, wo_sb, ones_sb,
                            cb_sb, mybir, gq="gpsimd")
                    cfgs = {(0, 0): (0, 1), (0, 1): (0, 1),
                            (1, 0): (0, 2), (1, 1): (0, 2),
                            (2, 0): (99, 1), (2, 1): (0, 1),
                            (3, 0): (2, 1), (3, 1): (2, 1)}
                    nxt2 = (order[qi + 1], 0) if qi + 1 < 4 else None
                    pf, strd = cfgs[(qi, 0)]
                    attention_segment(qc, 0, projq, nxt=(qc, 1),
                                      pop_from=pf, stride=strd)
                    pf, strd = cfgs[(qi, 1)]
                    attention_segment(qc, 1, projq, nxt=nxt2,
                                      pop_from=pf, stride=strd)
                    # drain leftover pieces between rounds (but let carried
                    # pieces spread into round order[1] first)
                    if qi:
                        while projq:
                            projq.pop(0)[1]()
                flush_pending()
                # proj of order[3] is carried into the next rep's early
                # attention rounds unless this is the last rep
                tail = _outproj_pieces(nc, P, d, order[3], ag_q, wo_sb,
                                       ones_sb, cb_sb, mybir,
                                       gq="gpsimd" if not is_last
                                       else "sync")
                if is_last:
                    for piece in tail:
                        piece[1]()
                    return []
                return tail


class Runner:
    """Builds + compiles once; callable repeatedly with per-core in_maps."""

    def __init__(self, mask_plan, reps=1):
        import jax
        from jax.sharding import Mesh, PartitionSpec
        from jax.experimental.shard_map import shard_map
        from concourse import mybir
        from concourse.bass2jax import (
            _bass_exec_p, install_neuronx_cc_hook, partition_id_tensor)

        self.jax = jax
        self.nc = _build(mask_plan, reps=reps)
        install_neuronx_cc_hook()
        nc = self.nc

        in_names, out_names, out_avals = [], [], []
        partition_name = (nc.partition_id_tensor.name
                          if nc.partition_id_tensor else None)
        for alloc in nc.m.functions[0].allocations:
            if not isinstance(alloc, mybir.MemoryLocationSet):
                continue
            name = alloc.memorylocations[0].name
            if alloc.kind == "ExternalInput":
                if name != partition_name:
                    in_names.append(name)
            elif alloc.kind == "ExternalOutput":
                out_names.append(name)
                shape = tuple(alloc.tensor_shape)
                dtype = mybir.dt.np(alloc.dtype)
                out_avals.append(jax.core.ShapedArray(shape, dtype))
        self.in_names = list(in_names)
        self.out_names = out_names
        self.out_avals = out_avals
        n_params = len(in_names)
        n_outs = len(out_avals)
        all_in_names = in_names + out_names
        if partition_name is not None:
            all_in_names.append(partition_name)
        donate = tuple(range(n_params, n_params + n_outs))

        def _body(*args):
            operands = list(args)
            if partition_name is not None:
                operands.append(partition_id_tensor())
            outs = _bass_exec_p.bind(
                *operands,
                out_avals=tuple(out_avals),
                in_names=tuple(all_in_names),
                out_names=tuple(out_names),
                lowering_input_output_aliases=(),
                sim_require_finite=True,
                sim_require_nnan=True,
                nc=nc,
            )
            return tuple(outs)

        devices = jax.devices()[:NCORES]
        mesh = Mesh(np.asarray(devices), ("core",))
        self.mesh = mesh
        in_specs = (PartitionSpec("core"),) * (n_params + n_outs)
        out_specs = (PartitionSpec("core"),) * n_outs
        self.fn = jax.jit(
            shard_map(_body, mesh=mesh, in_specs=in_specs,
                      out_specs=out_specs, check_rep=False),
            keep_unused=True)

    def prepare(self, in_maps):
        import jax
        from jax.sharding import NamedSharding, PartitionSpec
        sh = NamedSharding(self.mesh, PartitionSpec("core"))
        concat_in = [
            np.concatenate([np.asarray(m[name]) for m in in_maps], axis=0)
            for name in self.in_names
        ]
        self._dev_in = [jax.device_put(a, sh) for a in concat_in]
        jax.block_until_ready(self._dev_in)
        self._zero_sh = sh
        self._zcache = None

    def _zeros(self):
        import jax
        import jax.numpy as jnp
        if self._zcache is None:
            def mk(shape, dtype):
                return jax.jit(lambda: jnp.zeros(shape, dtype),
                               out_shardings=self._zero_sh)
            self._zcache = [
                mk((NCORES * a.shape[0], *a.shape[1:]), a.dtype)()
                for a in self.out_avals
            ]
            jax.block_until_ready(self._zcache)
        return self._zcache

    def run_prepared(self, fetch=True):
        import jax
        out = self.fn(*self._dev_in, *self._zeros())
        if not fetch:
            jax.block_until_ready(out)
            return None
        out = [np.asarray(o) for o in out]
        return [
            {name: out[i].reshape(NCORES, *self.out_avals[i].shape)[c]
             for i, name in enumerate(self.out_names)}
            for c in range(NCORES)
        ]

    def __call__(self, in_maps):
        self.prepare(in_maps)
        return self.run_prepared()

    def make_loop(self, n):
        """Jitted fn executing the NEFF n times sequentially (chained via a
        zeroed carry) inside one dispatch — for overhead-free timing."""
        import jax
        import jax.numpy as jnp
        from jax.sharding import PartitionSpec
        from jax.experimental.shard_map import shard_map
        from concourse.bass2jax import _bass_exec_p, partition_id_tensor

        nc = self.nc
        out_avals = self.out_avals
        in_names = self.in_names
        out_names = self.out_names
        partition_name = (nc.partition_id_tensor.name
                          if nc.partition_id_tensor else None)
        all_in_names = list(in_names) + list(out_names)
        if partition_name is not None:
            all_in_names.append(partition_name)

        def _loop(*args):
            ins = list(args)
            carry = [jnp.zeros(a.shape, a.dtype) for a in out_avals]
            outs = None
            for _ in range(n):
                operands = ins + carry
                if partition_name is not None:
                    operands.append(partition_id_tensor())
                outs = _bass_exec_p.bind(
                    *operands,
                    out_avals=tuple(out_avals),
                    in_names=tuple(all_in_names),
                    out_names=tuple(out_names),
                    lowering_input_output_aliases=(),
                    sim_require_finite=True,
                    sim_require_nnan=True,
                    nc=nc,
                )
                carry = [o * 0 for o in outs]
            return tuple(outs)

        n_params = len(in_names)
        in_specs = (PartitionSpec("core"),) * n_params
        out_specs = (PartitionSpec("core"),) * len(out_names)
        return jax.jit(shard_map(_loop, mesh=self.mesh, in_specs=in_specs,
                                 out_specs=out_specs, check_rep=False))

    def time_loop(self, n, iters=8):
        import time as _time
        import jax
        fn = self.make_loop(n)
        out = fn(*self._dev_in)
        jax.block_until_ready(out)
        ts = []
        for _ in range(iters):
            t0 = _time.perf_counter()
            out = fn(*self._dev_in)
            jax.block_until_ready(out)
            ts.append(_time.perf_counter() - t0)
        ts.sort()
        return ts[0], ts[len(ts) // 2]


def _mask_plan(maskT):
    """Classify each (kb 128-k-rows, qc 512-q-cols) block of the transposed
    mask. Per live block: (dead_prefix, strip_lo, strip_wid, strip_idx);
    NEG blocks are None. Build the compact strip tensor."""
    entries = [[None] * 4 for _ in range(16)]
    strips = []
    for qc in range(4):
        for kb in range(16):
            sub = maskT[kb * 128:(kb + 1) * 128, qc * 512:(qc + 1) * 512]
            if np.all(sub == 0.0):
                entries[kb][qc] = (0, 0, 0, -1)
                continue
            if np.all(sub <= -1e8):
                continue  # NEG: skip block entirely
            dead = np.all(sub <= -1e8, axis=0)   # fully-dead columns
            zero = np.all(sub == 0.0, axis=0)    # fully-zero columns
            dd = 0
            while dd < 512 and dead[dd]:
                dd += 1
            nz = np.nonzero(~zero)[0]
            nz = nz[nz >= dd]
            if len(nz) == 0:
                entries[kb][qc] = (dd, dd, 0, -1)
                continue
            lo, hi = int(nz[0]), int(nz[-1]) + 1
            if hi - lo <= 128:
                lo = min(lo, 512 - 128)
                wid = 128
            else:
                wid = hi - lo
            entries[kb][qc] = (dd, lo, wid, len(strips))
            strips.append(np.ascontiguousarray(sub[:, lo:lo + wid]))
    # PV accumulation starts full-width on the first live kb: its dead
    # prefix must be minimal per qc, else columns would be first-touched
    # with start=False. Degrade such (non-causal) qcs to untrimmed MIX.
    for qc in range(4):
        kbs = [kb for kb in range(16) if entries[kb][qc] is not None]
        if not kbs:
            continue
        dd0 = entries[kbs[0]][qc][0]
        if any(entries[kb][qc][0] < dd0 for kb in kbs):
            for kb in kbs:
                sub = maskT[kb * 128:(kb + 1) * 128,
                            qc * 512:(qc + 1) * 512]
                entries[kb][qc] = (0, 0, 512, len(strips))
                strips.append(np.ascontiguousarray(sub))
    maxw = max((s.shape[1] for s in strips), default=1)
    import ml_dtypes as _mldt
    if strips:
        maskm = np.zeros((len(strips), 128, maxw), dtype=_mldt.bfloat16)
        for i, s in enumerate(strips):
            maskm[i, :, :s.shape[1]] = s.astype(_mldt.bfloat16)
    else:
        maskm = np.zeros((1, 128, 1), dtype=_mldt.bfloat16)
    entries = tuple(tuple(r) for r in entries)
    return (entries, len(strips), maxw), maskm


def _host_prep(x, Wqkv_w, Wqkv_b, out_w, out_b, mask):
    """Build per-core in_maps + mask plan (numpy only)."""
    x2 = np.ascontiguousarray(np.asarray(x, dtype=np.float32)[0])   # [L, C]
    xT = x2.T                                                        # [C, L]
    import ml_dtypes as _mld
    xTc = np.ascontiguousarray(
        np.stack([xT[:, n * NW:(n + 1) * NW] for n in range(NCH)],
                 axis=0).astype(_mld.bfloat16))
    Wqkv_w = np.asarray(Wqkv_w, dtype=np.float32)
    Wqkv_b = np.asarray(Wqkv_b, dtype=np.float32)
    out_w = np.asarray(out_w, dtype=np.float32)
    out_b = np.asarray(out_b, dtype=np.float32)
    mask2 = np.asarray(mask, dtype=np.float32)[0, 0]                 # [L, L]
    maskT = np.ascontiguousarray(mask2.T)
    plan, maskm = _mask_plan(maskT)

    Wq, Wk, Wv = Wqkv_w[0:C], Wqkv_w[C:2 * C], Wqkv_w[2 * C:3 * C]
    bq, bk, bv = Wqkv_b[0:C], Wqkv_b[C:2 * C], Wqkv_b[2 * C:3 * C]

    import ml_dtypes
    pos = np.arange(L, dtype=np.float32)
    freq = np.exp(-np.arange(RD, dtype=np.float32) * (math.log(BASE) / RD))
    theta = pos[None, :] * freq[:, None]                             # [16, L]
    cos8 = np.ascontiguousarray(
        np.tile(np.cos(theta), (8, 1)).astype(ml_dtypes.bfloat16))
    sin8 = np.ascontiguousarray(
        np.tile(np.sin(theta), (8, 1)).astype(ml_dtypes.bfloat16))

    hidx = np.arange(HPC)[:, None]
    x1_idx = (80 * hidx + np.arange(RD)[None, :]).ravel()
    x2_idx = (80 * hidx + RD + np.arange(RD)[None, :]).ravel()
    rest_idx = (80 * hidx + ROT + np.arange(HD - ROT)[None, :]).ravel()

    ones128 = np.ones((1, 128), dtype=np.float32)
    vpad = np.zeros((128, 16, HPC, 48), dtype=ml_dtypes.bfloat16)
    vpad[..., 16] = 1.0

    in_maps = []
    for i in range(NCORES):
        rs = slice(DH * i, DH * (i + 1))
        Wq_i = Wq[rs] * SC
        bq_i = bq[rs] * SC
        Wk_i, bk_i, Wv_i = Wk[rs], bk[rs], Wv[rs]
        Wqk_i = np.concatenate([
            Wq_i[x1_idx], Wk_i[x1_idx],
            Wq_i[x2_idx], Wk_i[x2_idx],
            Wq_i[rest_idx], Wk_i[rest_idx]], axis=0)                 # [640, C]
        bqk_i = np.concatenate([
            bq_i[x1_idx], bk_i[x1_idx],
            bq_i[x2_idx], bk_i[x2_idx],
            bq_i[rest_idx], bk_i[rest_idx]], axis=0)
        wqk_t = np.ascontiguousarray(Wqk_i.T.astype(ml_dtypes.bfloat16))
        wv_t = np.ascontiguousarray(Wv_i.T.astype(ml_dtypes.bfloat16))
        Wo_i = out_w[rs]
        wo_t = np.ascontiguousarray(Wo_i.T.astype(ml_dtypes.bfloat16))
        cb_i = (out_b[rs] + Wo_i @ bv).astype(np.float32)[None, :]
        bqk_r = np.ascontiguousarray(bqk_i.reshape(5, 128).T)
        in_maps.append({
            "xTc": xTc,
            "wqk_t": wqk_t,
            "wv_t": wv_t,
            "wo_t": wo_t,
            "bqk": bqk_r,
            "cb": cb_i,
            "maskm": maskm,
            "cos8": cos8,
            "sin8": sin8,
            "ones128": ones128,
            "vpad": vpad,
        })
    return in_maps, plan


def get_runner(mask_plan, reps=1):
    key = (mask_plan, reps)
    if key not in _RUNNERS:
        _RUNNERS[key] = Runner(mask_plan, reps=reps)
    return _RUNNERS[key]


def kernel(x, Wqkv_w, Wqkv_b, out_w, out_b, mask):
    in_maps, plan = _host_prep(x, Wqkv_w, Wqkv_b, out_w, out_b, mask)
    runner = get_runner(plan)
    results = runner(in_maps)
    y = np.concatenate([results[i]["y"] for i in range(NCORES)], axis=1)
    return y.reshape(1, L, C)


# revision 69
# speedup vs baseline: 1.0957x; 1.0957x over previous
"""Phi-2-style attention layer (B=1, L=2048, D=2560, 32 heads, partial rope 32)
as a distributed Bass kernel on 8 TRN2 NeuronCores.

Sharding: tensor-parallel over heads (4 heads/core).
  - x is replicated, passed as contiguous 256-column chunks xTc [8, 2560, 256].
  - Each core computes rope'd qT/kT + v for its 4 heads, causal attention in
    the S^T (k-on-partitions) layout, then the normalized attention output
    attnT [320, 2048] is AllGathered to [2560, 2048]; each core computes its
    320-column slice of the output projection. Host concatenates the slices.

Attention phase (CD) is software-pipelined for PE occupancy:
  - per (qc, head-pair): S^T for both heads goes into one [128, 2, 512] PSUM
    mega-tile (2 banks); one batched exp per kb covers both heads.
  - score/PV matmuls are trimmed to live columns (true-causal): for a
    diagonal block only cols >= c0 are computed, and the additive mask is
    applied on a 128-wide strip only (DVE) with a separate small exp.
  - exp output (est) and V are bf16; denominator comes from a ones-column at
    padded V column 96.
  - out-proj for round qc is deferred two rounds so its AllGather hides
    behind later rounds' attention matmuls.

All matmuls run in bf16 (f32r for tiny bias rank-1), fp32 PSUM accumulation.
Softmax is computed unnormalized (scores are O(5); exp without
max-subtraction is safe; mask -1e9 underflows exp to exactly 0).
"""

import math
from contextlib import ExitStack

import numpy as np

L = 2048
C = 2560
NCORES = 8
HPC = 4          # heads per core
HD = 80          # head dim
DH = HPC * HD    # 320 dims per core
ROT = 32
RD = ROT // 2    # 16
BASE = 10000.0
KT = C // 128    # 20 k tiles
NW = 256         # x chunk width
NCH = L // NW    # 8 chunks
SC = 1.0 / math.sqrt(HD)

_RUNNERS = {}


def _build(mask_plan, reps=1, sim_mode=False):
    import concourse.bacc as bacc
    import concourse.tile as tile
    from concourse import mybir

    f32 = mybir.dt.float32
    f32r = mybir.dt.float32r
    entries, nmix, maxw = mask_plan

    nc = bacc.Bacc("TRN2", target_bir_lowering=False, debug=False,
                   num_devices=NCORES)

    d = {}
    bf16_ = mybir.dt.bfloat16
    d["xTc"] = nc.dram_tensor("xTc", [NCH, C, NW], bf16_, kind="ExternalInput").ap()
    d["wqk"] = nc.dram_tensor("wqk_t", [C, 2 * DH], bf16_, kind="ExternalInput").ap()
    d["wv"] = nc.dram_tensor("wv_t", [C, DH], bf16_, kind="ExternalInput").ap()
    d["wo"] = nc.dram_tensor("wo_t", [C, DH], bf16_, kind="ExternalInput").ap()
    d["bqk"] = nc.dram_tensor("bqk", [128, 5], f32, kind="ExternalInput").ap()
    d["cb"] = nc.dram_tensor("cb", [1, DH], f32r, kind="ExternalInput").ap()
    d["maskm"] = nc.dram_tensor("maskm", [max(nmix, 1), 128, max(maxw, 1)],
                                bf16_, kind="ExternalInput").ap()
    d["cos"] = nc.dram_tensor("cos8", [128, L], bf16_, kind="ExternalInput").ap()
    d["sin"] = nc.dram_tensor("sin8", [128, L], bf16_, kind="ExternalInput").ap()
    d["ones"] = nc.dram_tensor("ones128", [1, 128], f32r, kind="ExternalInput").ap()
    d["vpad"] = nc.dram_tensor("vpad", [128, 16, HPC, 48], bf16_,
                               kind="ExternalInput").ap()
    d["y"] = nc.dram_tensor("y", [L, DH], f32, kind="ExternalOutput").ap()

    with tile.TileContext(nc) as tc:
        with ExitStack() as gctx:
            P = {
                "small": gctx.enter_context(
                    tc.tile_pool(name="smallg", bufs=2)),
                "wo": gctx.enter_context(tc.tile_pool(name="wog", bufs=2)),
                "g": gctx.enter_context(tc.tile_pool(name="gg", bufs=2)),
                "dram": gctx.enter_context(
                    tc.tile_pool(name="dramg", bufs=1, space="DRAM")),
            }
            carry = []
            for rep in range(reps):
                carry = _build_rep(nc, tc, mybir, f32, f32r, d, mask_plan,
                                   rep, sim_mode, P, carry,
                                   is_last=(rep == reps - 1))
    nc.compile()
    return nc


def _rest_runs(mi):
    """For qk m-tile mi in {2,3,4}: contiguous runs (is_q, head, dst_row,
    src_row, nrows) mapping eviction rows to per-head tiles."""
    runs = []
    p = 0
    while p < 128:
        r = (mi - 2) * 128 + p
        if r < 192:
            h, j, is_q = r // 48, r % 48, True
        else:
            h, j, is_q = (r - 192) // 48, (r - 192) % 48, False
        n = min(48 - j, 128 - p)
        runs.append((is_q, h, 32 + j, p, n))
        p += n
    return runs


def _outproj_pieces(nc, P, d, qc, ag_q, wo_sb, ones_sb, cb_sb, mybir,
                    gq="sync"):
    """Out-projection for one 512-wide q round, split into small pieces
    (one g-load + 8 matmul chunks) that callers interleave into attention
    kb-loops as PE filler. PSUM/eviction tiles come from the pools of the
    rep that RUNS the piece (P["psD_cur"]/P["ye_cur"]), so pieces may be
    carried across rep boundaries; g/wo/ones/cb data tiles are captured
    from the building rep (persistent pools, bufs=2)."""
    f32 = mybir.dt.float32
    bf16 = mybir.dt.bfloat16
    g = P["g"].tile([128, KT, 512], bf16, name="g", tag="g")

    def gload():
        eng = {"gpsimd": nc.gpsimd, "scalar": nc.scalar,
               "sync": nc.sync}[gq]
        ag_r = ag_q[qc].rearrange("(t p) w -> p t w", p=128)
        eng.dma_start(out=g[:, 0:10, :], in_=ag_r[:, 0:10, :])
        eng.dma_start(out=g[:, 10:KT, :], in_=ag_r[:, 10:KT, :])

    pieces = [("g", gload)]
    psys = {}
    for jj in range(4):
        lb = qc * 4 + jj
        for kc in range(4):
            def pk(jj=jj, lb=lb, kc=kc):
                if kc == 0:
                    psy = P["psD_cur"].tile([128, DH], f32, name="psy",
                                            tag="psy")
                    psys[jj] = psy
                else:
                    psy = psys[jj]
                for k in range(5 * kc, 5 * kc + 5):
                    nc.tensor.matmul(psy[:],
                                     g[:, k, jj * 128:(jj + 1) * 128],
                                     wo_sb[:, k, :],
                                     start=(k == 0), stop=False)
                if kc == 3:
                    psys.pop(jj)
                    nc.tensor.matmul(psy[:], ones_sb[:], cb_sb[:],
                                     start=False, stop=True)
                    ye = P["ye_cur"].tile([128, DH], f32, name="ye",
                                          tag="ye")
                    nc.vector.tensor_scalar_add(ye[:], psy[:], 0.0)
                    nc.gpsimd.dma_start(
                        out=d["y"][lb * 128:(lb + 1) * 128, :], in_=ye[:])

            pieces.append(("mm", pk))
    return pieces


def _build_rep(nc, tc, mybir, f32, f32r, d, mask_plan, rep, sim_mode, P,
               carry, is_last):
    Exp = mybir.ActivationFunctionType.Exp
    Ln = mybir.ActivationFunctionType.Ln
    Identity = mybir.ActivationFunctionType.Identity
    Copy = mybir.ActivationFunctionType.Copy
    bypass = mybir.AluOpType.bypass
    add_op = mybir.AluOpType.add
    bf16 = mybir.dt.bfloat16
    entries, nmix, maxw = mask_plan

    wqk_r = d["wqk"].rearrange("(t p) m -> p t m", p=128)
    wv_r = d["wv"].rearrange("(t p) m -> p t m", p=128)
    wo_r = d["wo"].rearrange("(t p) m -> p t m", p=128)

    live_kb = {qc: [kb for kb in range(16) if entries[kb][qc] is not None]
               for qc in range(4)}

    with ExitStack() as ctx:
        small = P["small"]
        ones_sb = small.tile([1, 128], f32r, name="ones", tag="ones")
        nc.scalar.dma_start(out=ones_sb[:], in_=d["ones"][:])
        cb_sb = small.tile([1, DH], f32r, name="cb", tag="cb")
        nc.scalar.dma_start(out=cb_sb[:], in_=d["cb"][:])
        bqk_sb = small.tile([128, 5], f32, name="bqk", tag="bqk")
        nc.scalar.dma_start(out=bqk_sb[:], in_=d["bqk"][:])
        dpool = P["dram"]
        bounce_q = [dpool.tile([DH, 512], bf16, name=f"bnc{qc}",
                               tag=f"bnc{qc}") for qc in range(4)]
        ag_q = [dpool.tile([C, 512], bf16, name=f"agq{qc}", tag=f"agq{qc}",
                           addr_space="Local" if sim_mode else "Shared")
                for qc in range(4)]

        with ExitStack() as qctx:  # phases AB + CD
            qkv_pool = qctx.enter_context(
                tc.tile_pool(name=f"qkv{rep}", bufs=1))
            qh = [qkv_pool.tile([128, L], bf16, name=f"qh{h}", tag=f"qh{h}")
                  for h in range(HPC)]
            kh = [qkv_pool.tile([128, L], bf16, name=f"kh{h}", tag=f"kh{h}")
                  for h in range(HPC)]
            v_sb = qkv_pool.tile([128, 16, HPC, 128], bf16, name="v", tag="v")
            nc.gpsimd.dma_start(out=v_sb[:, :, :, HD:128], in_=d["vpad"][:])

            # ---------- Phase AB: full QKV projection + rope + scatter ------
            with ExitStack() as actx:
                cpool = actx.enter_context(
                    tc.tile_pool(name=f"cs{rep}", bufs=1))
                cos_sb = cpool.tile([128, L], bf16, name="cos", tag="cos")
                sin_sb = cpool.tile([128, L], bf16, name="sin", tag="sin")
                nc.scalar.dma_start(out=cos_sb[:], in_=d["cos"][:])
                nc.scalar.dma_start(out=sin_sb[:], in_=d["sin"][:])
                s1f = cpool.tile([128, L], bf16, name="s1f", tag="s1f")
                s2f = cpool.tile([128, L], bf16, name="s2f", tag="s2f")
                wpool = actx.enter_context(
                    tc.tile_pool(name=f"wab{rep}", bufs=1))
                xpool = actx.enter_context(
                    tc.tile_pool(name=f"xab{rep}", bufs=2))
                epool = actx.enter_context(
                    tc.tile_pool(name=f"evab{rep}", bufs=2))
                psA = actx.enter_context(
                    tc.tile_pool(name=f"psab{rep}", bufs=1, space="PSUM"))

                wqk_sb = wpool.tile([128, KT, 2 * DH], bf16, name="wqk",
                                    tag="wqk")
                wv_sb = wpool.tile([128, KT, DH], bf16, name="wv", tag="wv")
                # split by k-half so khf=0 matmuls start after half the
                # bytes; on sync: the sync engine is idle at the end of the
                # previous rep's CD, so these run during it (the scalar
                # engine is stuck behind that rep's exp stream)
                nc.sync.dma_start(out=wqk_sb[:, 0:10, :],
                                  in_=wqk_r[:, 0:10, :])
                nc.sync.dma_start(out=wv_sb[:, 0:10, :],
                                  in_=wv_r[:, 0:10, :])
                nc.sync.dma_start(out=wqk_sb[:, 10:20, :],
                                  in_=wqk_r[:, 10:20, :])
                nc.sync.dma_start(out=wv_sb[:, 10:20, :],
                                  in_=wv_r[:, 10:20, :])

                for n in range(NCH):
                    nsl = slice(n * NW, (n + 1) * NW)
                    psm = [psA.tile([128, NW], f32, name=f"psm{m}",
                                    tag=f"psm{m}") for m in range(5)]
                    psv = [psA.tile([128, DH], f32, name=f"psv{j}",
                                    tag=f"psv{j}") for j in range(2)]
                    for khf in range(2):
                        xa = xpool.tile([128, 10, NW], bf16, name="xa",
                                        tag="xa")
                        nc.sync.dma_start(
                            out=xa[:],
                            in_=d["xTc"][n, khf * 1280:(khf + 1) * 1280, :]
                            .rearrange("(t p) w -> p t w", p=128))
                        for m in range(5):
                            for kk in range(10):
                                k = khf * 10 + kk
                                nc.tensor.matmul(
                                    psm[m][:],
                                    wqk_sb[:, k, m * 128:(m + 1) * 128],
                                    xa[:, kk, :],
                                    start=(k == 0), stop=(k == KT - 1))
                        for j in range(2):
                            for kk in range(10):
                                k = khf * 10 + kk
                                nc.tensor.matmul(
                                    psv[j][:],
                                    xa[:, kk, j * 128:(j + 1) * 128],
                                    wv_sb[:, k, :],
                                    start=(k == 0), stop=(k == KT - 1))
                    # evictions
                    ev = {}
                    for m in range(5):
                        e = epool.tile([128, NW], bf16, name=f"e{m}",
                                       tag=f"e{m}")
                        nc.scalar.activation(e[:], psm[m][:], Identity,
                                             bias=bqk_sb[:, m:m + 1])
                        ev[m] = e
                    for j in range(2):
                        nc.scalar.activation(
                            v_sb[:, 2 * n + j, :, 0:HD],
                            psv[j][:].rearrange("p (h dd) -> p h dd", h=HPC),
                            Copy)
                    # rope chunk
                    co, si = cos_sb[:, nsl], sin_sb[:, nsl]
                    rt1 = epool.tile([128, NW], f32, name="rt1", tag="rt1")
                    rt2 = epool.tile([128, NW], f32, name="rt2", tag="rt2")
                    nc.vector.tensor_mul(rt1[:], ev[0][:], co)
                    nc.vector.tensor_mul(rt2[:], ev[1][:], si)
                    nc.vector.tensor_sub(s1f[:, nsl], rt1[:], rt2[:])
                    nc.vector.tensor_mul(rt1[:], ev[0][:], si)
                    nc.vector.tensor_mul(rt2[:], ev[1][:], co)
                    nc.vector.tensor_add(s2f[:, nsl], rt1[:], rt2[:])
                    # scatter rest rows of m=2,3,4
                    for m in (2, 3, 4):
                        for is_q, h, dr, sr, nr in _rest_runs(m):
                            dst = qh[h] if is_q else kh[h]
                            nc.scalar.dma_start(
                                out=dst[dr:dr + nr, nsl],
                                in_=ev[m][sr:sr + nr, :])

                for h in range(HPC):
                    nc.gpsimd.dma_start(out=qh[h][0:16, :],
                                        in_=s1f[16 * h:16 * h + 16, :])
                    nc.sync.dma_start(out=qh[h][16:32, :],
                                      in_=s2f[16 * h:16 * h + 16, :])
                    nc.gpsimd.dma_start(out=kh[h][0:16, :],
                                        in_=s1f[64 + 16 * h:80 + 16 * h, :])
                    nc.sync.dma_start(out=kh[h][16:32, :],
                                      in_=s2f[64 + 16 * h:80 + 16 * h, :])

            # ---------- Phase CD: attention + per-qc AllGather + out-proj ---
            with ExitStack() as cctx:
                mpool = cctx.enter_context(
                    tc.tile_pool(name=f"maskp{rep}", bufs=1))
                epool = cctx.enter_context(
                    tc.tile_pool(name=f"est{rep}", bufs=3))
                smpool = cctx.enter_context(
                    tc.tile_pool(name=f"smp{rep}", bufs=2))
                atpool = cctx.enter_context(
                    tc.tile_pool(name=f"attnq{rep}", bufs=2))
                npool = cctx.enter_context(
                    tc.tile_pool(name=f"norm{rep}", bufs=2))
                ypool = cctx.enter_context(
                    tc.tile_pool(name=f"ye{rep}", bufs=2))
                ps_st = cctx.enter_context(
                    tc.tile_pool(name=f"psST{rep}", bufs=2, space="PSUM"))
                ps_pv = cctx.enter_context(
                    tc.tile_pool(name=f"psPV{rep}", bufs=1, space="PSUM"))
                psD = cctx.enter_context(
                    tc.tile_pool(name=f"psD{rep}", bufs=2, space="PSUM"))

                pvcpool = cctx.enter_context(
                    tc.tile_pool(name=f"pvc{rep}", bufs=2))
                P["psD_cur"] = psD
                P["ye_cur"] = ypool

                # compact mask strips, loaded once
                mq = None
                if nmix:
                    mq = mpool.tile([128, nmix, maxw], bf16, name="mq",
                                    tag="mq")
                    nc.sync.dma_start(out=mq[:], in_=d["maskm"]
                                      .rearrange("b p w -> p b w"))
                wo_sb = P["wo"].tile([128, KT, DH], bf16, name="wo",
                                     tag="wo")
                nc.sync.dma_start(out=wo_sb[:], in_=wo_r[:])

                # deferred normalize/bounce/collective work: emitted a few
                # kb-iterations into the NEXT segment so slow per-head chains
                # never head-of-line-block the attention engines
                pending = []

                def flush_pending():
                    for f in pending:
                        f()
                    pending.clear()

                def mk_norm(qc, hp, pvc, lnd):
                    def f():
                        for j in range(2):
                            h = 2 * hp + j
                            den = npool.tile([1, 512], f32, name="den",
                                             tag="den")
                            nc.scalar.activation(den[:], lnd[j][:],
                                                 Exp, scale=-1.0)
                            denb = npool.tile([HD, 512], f32, name="denb",
                                              tag="denb")
                            nc.gpsimd.partition_broadcast(denb[:], den[:])
                            attnq = atpool.tile([HD, 512], bf16,
                                                name="attnq", tag="attnq")
                            nc.vector.tensor_mul(attnq[:],
                                                 pvc[0:HD, j, :], denb[:])
                            nc.gpsimd.dma_start(
                                out=bounce_q[qc][h * HD:(h + 1) * HD, :],
                                in_=attnq[:])
                        if hp == 0:
                            return
                        if sim_mode:
                            for rr in range(NCORES):
                                nc.gpsimd.dma_start(
                                    out=ag_q[qc][rr * DH:(rr + 1) * DH, :],
                                    in_=bounce_q[qc][:])
                        else:
                            nc.gpsimd.collective_compute(
                                "AllGather",
                                bypass,
                                replica_groups=[list(range(NCORES))],
                                ins=[bounce_q[qc].opt()],
                                outs=[ag_q[qc].opt()],
                            )
                    return f

                # st tiles are emitted two work-items ahead of their exp/pv
                # consumers, ACROSS segment boundaries, so the PE never sits
                # behind a cold exp pipeline at a segment start
                sts = {}

                def emit_st(qc, hp, kb):
                    dd, lo, wid, sidx = entries[kb][qc]
                    st = ps_st.tile([128, 2, 512], f32, name="st", tag="st")
                    for j in range(2):
                        h = 2 * hp + j
                        nc.tensor.matmul(
                            st[:, j, dd:512],
                            kh[h][0:HD, kb * 128:(kb + 1) * 128],
                            qh[h][0:HD, qc * 512 + dd:qc * 512 + 512],
                            start=True, stop=True)
                    sts[(qc, hp, kb)] = st

                def attention_segment(qc, hp, projq, nxt=None,
                                      pop_from=3, stride=1):
                    kbs = live_kb[qc]
                    if projq and projq[0][0] == "g":
                        projq.pop(0)[1]()
                    pv = [ps_pv.tile([128, 512], f32, name=f"pv{j}",
                                     tag=f"pv{j}") for j in range(2)]
                    for kb in kbs[:2]:
                        if (qc, hp, kb) not in sts:
                            emit_st(qc, hp, kb)
                    # pure-PE filler at segment entry covers the latency of
                    # the first exp before the first pv can issue
                    if pop_from == 0:
                        for _ in range(2):
                            if projq and projq[0][0] == "mm":
                                projq.pop(0)[1]()
                    for i, kb in enumerate(kbs):
                        if i + 2 < len(kbs):
                            emit_st(qc, hp, kbs[i + 2])
                        elif nxt is not None:
                            nqc, nhp = nxt
                            nkbs = live_kb[nqc]
                            k = i + 2 - len(kbs)
                            if k < min(2, len(nkbs)):
                                emit_st(nqc, nhp, nkbs[k])
                        # out-proj matmul pieces as PE filler while the
                        # scalar engine works through the exps
                        if (i >= pop_from and (i - pop_from) % stride == 0
                                and projq):
                            projq.pop(0)[1]()
                        if i == min(2, len(kbs) - 1):
                            flush_pending()
                        dd, lo, wid, sidx = entries[kb][qc]
                        st = sts.pop((qc, hp, kb))
                        est = epool.tile([128, 2, 512], bf16, name="est",
                                         tag="est")
                        if wid:
                            hi = lo + wid
                            sm = smpool.tile([128, 2, maxw], f32,
                                             name="sm", tag="sm")
                            for j in range(2):
                                nc.vector.scalar_tensor_tensor(
                                    out=sm[:, j, 0:wid],
                                    in0=st[:, j, lo:hi], scalar=1.0,
                                    in1=mq[:, sidx, 0:wid],
                                    op0=bypass, op1=add_op)
                            if lo > dd:
                                nc.scalar.activation(
                                    est[:, :, dd:lo], st[:, :, dd:lo], Exp)
                            if hi < 512:
                                nc.scalar.activation(
                                    est[:, :, hi:512], st[:, :, hi:512],
                                    Exp)
                            nc.scalar.activation(
                                est[:, :, lo:hi], sm[:, :, 0:wid], Exp)
                        else:
                            nc.scalar.activation(
                                est[:, :, dd:512], st[:, :, dd:512], Exp)
                        for j in range(2):
                            h = 2 * hp + j
                            nc.tensor.matmul(
                                pv[j][:, dd:512],
                                v_sb[:, kb, h, :],
                                est[:, j, dd:512],
                                start=(i == 0),
                                stop=(i == len(kbs) - 1))
                    # release PSUM fast: DVE evicts pv, Act takes ln of the
                    # denominator rows; the rest of the normalize chain is
                    # deferred via `pending`
                    pvc = pvcpool.tile([128, 2, 512], f32, name="pvc",
                                       tag="pvc")
                    lnd = [npool.tile([1, 512], f32, name=f"lnd{j}",
                                      tag=f"lnd{j}") for j in range(2)]
                    for j in range(2):
                        nc.vector.tensor_scalar_add(pvc[:, j, :],
                                                    pv[j][:], 0.0)
                        nc.scalar.activation(lnd[j][:],
                                             pv[j][96:97, :], Ln)
                    pending.append(mk_norm(qc, hp, pvc, lnd))

                order = (0, 1, 2, 3)
                projq = list(carry)
                for qi, qc in enumerate(order):
                    if qi >= 2:
                        # proj(order[1])'s g-load rides the scalar engine:
                        # it reaches that instruction just as AG completes,
                        # keeping the sync queue clear for the next rep's
                        # weight prefetch
                        projq += _outproj_pieces(
                            nc, P, d, order[qi - 2], ag_q, wo_sb, ones_sb,
                            cb_sb, mybir,
                            gq="scalar" if qi == 3 else "sync")
                    if qi == 3:
                        # proj of order[2] also rides inside round order[3]
                        # (second head-pair) instead of trailing the rep;
                        # its g-load goes via gpsimd so the sync queue is
                        # free for the next rep's weight/x prefetches
                        projq += _outproj_pieces(
                            nc, P, d, order[2], ag_q, wo_sb, ones_sb,
                            cb_sb, mybir, gq="gpsimd")
                    cfgs = {(0, 0): (0, 1), (0, 1): (0, 1),
                            (1, 0): (0, 2), (1, 1): (0, 2),
                            (2, 0): (99, 1), (2, 1): (3, 1),
                            (3, 0): (2, 1), (3, 1): (2, 1)}
                    nxt2 = (order[qi + 1], 0) if qi + 1 < 4 else None
                    pf, strd = cfgs[(qi, 0)]
                    attention_segment(qc, 0, projq, nxt=(qc, 1),
                                      pop_from=pf, stride=strd)
                    pf, strd = cfgs[(qi, 1)]
                    attention_segment(qc, 1, projq, nxt=nxt2,
                                      pop_from=pf, stride=strd)
                    if qi == 0:
                        # round 0's AllGather is on the critical path of
                        # proj(0) two rounds later: kick it off inline
                        # instead of deferring into round 1
                        flush_pending()
                    # drain leftover pieces between rounds (but let carried
                    # pieces spread into round order[1] first)
                    if qi:
                        while projq:
                            projq.pop(0)[1]()
                flush_pending()
                # proj of order[3] is carried into the next rep's early
                # attention rounds unless this is the last rep
                tail = _outproj_pieces(nc, P, d, order[3], ag_q, wo_sb,
                                       ones_sb, cb_sb, mybir,
                                       gq="gpsimd" if not is_last
                                       else "sync")
                if is_last:
                    for piece in tail:
                        piece[1]()
                    return []
                return tail


class Runner:
    """Builds + compiles once; callable repeatedly with per-core in_maps."""

    def __init__(self, mask_plan, reps=1):
        import jax
        from jax.sharding import Mesh, PartitionSpec
        from jax.experimental.shard_map import shard_map
        from concourse import mybir
        from concourse.bass2jax import (
            _bass_exec_p, install_neuronx_cc_hook, partition_id_tensor)

        self.jax = jax
        self.nc = _build(mask_plan, reps=reps)
        install_neuronx_cc_hook()
        nc = self.nc

        in_names, out_names, out_avals = [], [], []
        partition_name = (nc.partition_id_tensor.name
                          if nc.partition_id_tensor else None)
        for alloc in nc.m.functions[0].allocations:
            if not isinstance(alloc, mybir.MemoryLocationSet):
                continue
            name = alloc.memorylocations[0].name
            if alloc.kind == "ExternalInput":
                if name != partition_name:
                    in_names.append(name)
            elif alloc.kind == "ExternalOutput":
                out_names.append(name)
                shape = tuple(alloc.tensor_shape)
                dtype = mybir.dt.np(alloc.dtype)
                out_avals.append(jax.core.ShapedArray(shape, dtype))
        self.in_names = list(in_names)
        self.out_names = out_names
        self.out_avals = out_avals
        n_params = len(in_names)
        n_outs = len(out_avals)
        all_in_names = in_names + out_names
        if partition_name is not None:
            all_in_names.append(partition_name)
        donate = tuple(range(n_params, n_params + n_outs))

        def _body(*args):
            operands = list(args)
            if partition_name is not None:
                operands.append(partition_id_tensor())
            outs = _bass_exec_p.bind(
                *operands,
                out_avals=tuple(out_avals),
                in_names=tuple(all_in_names),
                out_names=tuple(out_names),
                lowering_input_output_aliases=(),
                sim_require_finite=True,
                sim_require_nnan=True,
                nc=nc,
            )
            return tuple(outs)

        devices = jax.devices()[:NCORES]
        mesh = Mesh(np.asarray(devices), ("core",))
        self.mesh = mesh
        in_specs = (PartitionSpec("core"),) * (n_params + n_outs)
        out_specs = (PartitionSpec("core"),) * n_outs
        self.fn = jax.jit(
            shard_map(_body, mesh=mesh, in_specs=in_specs,
                      out_specs=out_specs, check_rep=False),
            keep_unused=True)

    def prepare(self, in_maps):
        import jax
        from jax.sharding import NamedSharding, PartitionSpec
        sh = NamedSharding(self.mesh, PartitionSpec("core"))
        concat_in = [
            np.concatenate([np.asarray(m[name]) for m in in_maps], axis=0)
            for name in self.in_names
        ]
        self._dev_in = [jax.device_put(a, sh) for a in concat_in]
        jax.block_until_ready(self._dev_in)
        self._zero_sh = sh
        self._zcache = None

    def _zeros(self):
        import jax
        import jax.numpy as jnp
        if self._zcache is None:
            def mk(shape, dtype):
                return jax.jit(lambda: jnp.zeros(shape, dtype),
                               out_shardings=self._zero_sh)
            self._zcache = [
                mk((NCORES * a.shape[0], *a.shape[1:]), a.dtype)()
                for a in self.out_avals
            ]
            jax.block_until_ready(self._zcache)
        return self._zcache

    def run_prepared(self, fetch=True):
        import jax
        out = self.fn(*self._dev_in, *self._zeros())
        if not fetch:
            jax.block_until_ready(out)
            return None
        out = [np.asarray(o) for o in out]
        return [
            {name: out[i].reshape(NCORES, *self.out_avals[i].shape)[c]
             for i, name in enumerate(self.out_names)}
            for c in range(NCORES)
        ]

    def __call__(self, in_maps):
        self.prepare(in_maps)
        return self.run_prepared()

    def make_loop(self, n):
        """Jitted fn executing the NEFF n times sequentially (chained via a
        zeroed carry) inside one dispatch — for overhead-free timing."""
        import jax
        import jax.numpy as jnp
        from jax.sharding import PartitionSpec
        from jax.experimental.shard_map import shard_map
        from concourse.bass2jax import _bass_exec_p, partition_id_tensor

        nc = self.nc
        out_avals = self.out_avals
        in_names = self.in_names
        out_names = self.out_names
        partition_name = (nc.partition_id_tensor.name
                          if nc.partition_id_tensor else None)
        all_in_names = list(in_names) + list(out_names)
        if partition_name is not None:
            all_in_names.append(partition_name)

        def _loop(*args):
            ins = list(args)
            carry = [jnp.zeros(a.shape, a.dtype) for a in out_avals]
            outs = None
            for _ in range(n):
                operands = ins + carry
                if partition_name is not None:
                    operands.append(partition_id_tensor())
                outs = _bass_exec_p.bind(
                    *operands,
                    out_avals=tuple(out_avals),
                    in_names=tuple(all_in_names),
                    out_names=tuple(out_names),
                    lowering_input_output_aliases=(),
                    sim_require_finite=True,
                    sim_require_nnan=True,
                    nc=nc,
                )
                carry = [o * 0 for o in outs]
            return tuple(outs)

        n_params = len(in_names)
        in_specs = (PartitionSpec("core"),) * n_params
        out_specs = (PartitionSpec("core"),) * len(out_names)
        return jax.jit(shard_map(_loop, mesh=self.mesh, in_specs=in_specs,
                                 out_specs=out_specs, check_rep=False))

    def time_loop(self, n, iters=8):
        import time as _time
        import jax
        fn = self.make_loop(n)
        out = fn(*self._dev_in)
        jax.block_until_ready(out)
        ts = []
        for _ in range(iters):
            t0 = _time.perf_counter()
            out = fn(*self._dev_in)
            jax.block_until_ready(out)
            ts.append(_time.perf_counter() - t0)
        ts.sort()
        return ts[0], ts[len(ts) // 2]


def _mask_plan(maskT):
    """Classify each (kb 128-k-rows, qc 512-q-cols) block of the transposed
    mask. Per live block: (dead_prefix, strip_lo, strip_wid, strip_idx);
    NEG blocks are None. Build the compact strip tensor."""
    entries = [[None] * 4 for _ in range(16)]
    strips = []
    for qc in range(4):
        for kb in range(16):
            sub = maskT[kb * 128:(kb + 1) * 128, qc * 512:(qc + 1) * 512]
            if np.all(sub == 0.0):
                entries[kb][qc] = (0, 0, 0, -1)
                continue
            if np.all(sub <= -1e8):
                continue  # NEG: skip block entirely
            dead = np.all(sub <= -1e8, axis=0)   # fully-dead columns
            zero = np.all(sub == 0.0, axis=0)    # fully-zero columns
            dd = 0
            while dd < 512 and dead[dd]:
                dd += 1
            nz = np.nonzero(~zero)[0]
            nz = nz[nz >= dd]
            if len(nz) == 0:
                entries[kb][qc] = (dd, dd, 0, -1)
                continue
            lo, hi = int(nz[0]), int(nz[-1]) + 1
            if hi - lo <= 128:
                lo = min(lo, 512 - 128)
                wid = 128
            else:
                wid = hi - lo
            entries[kb][qc] = (dd, lo, wid, len(strips))
            strips.append(np.ascontiguousarray(sub[:, lo:lo + wid]))
    # PV accumulation starts full-width on the first live kb: its dead
    # prefix must be minimal per qc, else columns would be first-touched
    # with start=False. Degrade such (non-causal) qcs to untrimmed MIX.
    for qc in range(4):
        kbs = [kb for kb in range(16) if entries[kb][qc] is not None]
        if not kbs:
            continue
        dd0 = entries[kbs[0]][qc][0]
        if any(entries[kb][qc][0] < dd0 for kb in kbs):
            for kb in kbs:
                sub = maskT[kb * 128:(kb + 1) * 128,
                            qc * 512:(qc + 1) * 512]
                entries[kb][qc] = (0, 0, 512, len(strips))
                strips.append(np.ascontiguousarray(sub))
    maxw = max((s.shape[1] for s in strips), default=1)
    import ml_dtypes as _mldt
    if strips:
        maskm = np.zeros((len(strips), 128, maxw), dtype=_mldt.bfloat16)
        for i, s in enumerate(strips):
            maskm[i, :, :s.shape[1]] = s.astype(_mldt.bfloat16)
    else:
        maskm = np.zeros((1, 128, 1), dtype=_mldt.bfloat16)
    entries = tuple(tuple(r) for r in entries)
    return (entries, len(strips), maxw), maskm


def _host_prep(x, Wqkv_w, Wqkv_b, out_w, out_b, mask):
    """Build per-core in_maps + mask plan (numpy only)."""
    x2 = np.ascontiguousarray(np.asarray(x, dtype=np.float32)[0])   # [L, C]
    xT = x2.T                                                        # [C, L]
    import ml_dtypes as _mld
    xTc = np.ascontiguousarray(
        np.stack([xT[:, n * NW:(n + 1) * NW] for n in range(NCH)],
                 axis=0).astype(_mld.bfloat16))
    Wqkv_w = np.asarray(Wqkv_w, dtype=np.float32)
    Wqkv_b = np.asarray(Wqkv_b, dtype=np.float32)
    out_w = np.asarray(out_w, dtype=np.float32)
    out_b = np.asarray(out_b, dtype=np.float32)
    mask2 = np.asarray(mask, dtype=np.float32)[0, 0]                 # [L, L]
    maskT = np.ascontiguousarray(mask2.T)
    plan, maskm = _mask_plan(maskT)

    Wq, Wk, Wv = Wqkv_w[0:C], Wqkv_w[C:2 * C], Wqkv_w[2 * C:3 * C]
    bq, bk, bv = Wqkv_b[0:C], Wqkv_b[C:2 * C], Wqkv_b[2 * C:3 * C]

    import ml_dtypes
    pos = np.arange(L, dtype=np.float32)
    freq = np.exp(-np.arange(RD, dtype=np.float32) * (math.log(BASE) / RD))
    theta = pos[None, :] * freq[:, None]                             # [16, L]
    cos8 = np.ascontiguousarray(
        np.tile(np.cos(theta), (8, 1)).astype(ml_dtypes.bfloat16))
    sin8 = np.ascontiguousarray(
        np.tile(np.sin(theta), (8, 1)).astype(ml_dtypes.bfloat16))

    hidx = np.arange(HPC)[:, None]
    x1_idx = (80 * hidx + np.arange(RD)[None, :]).ravel()
    x2_idx = (80 * hidx + RD + np.arange(RD)[None, :]).ravel()
    rest_idx = (80 * hidx + ROT + np.arange(HD - ROT)[None, :]).ravel()

    ones128 = np.ones((1, 128), dtype=np.float32)
    vpad = np.zeros((128, 16, HPC, 48), dtype=ml_dtypes.bfloat16)
    vpad[..., 16] = 1.0

    in_maps = []
    for i in range(NCORES):
        rs = slice(DH * i, DH * (i + 1))
        Wq_i = Wq[rs] * SC
        bq_i = bq[rs] * SC
        Wk_i, bk_i, Wv_i = Wk[rs], bk[rs], Wv[rs]
        Wqk_i = np.concatenate([
            Wq_i[x1_idx], Wk_i[x1_idx],
            Wq_i[x2_idx], Wk_i[x2_idx],
            Wq_i[rest_idx], Wk_i[rest_idx]], axis=0)                 # [640, C]
        bqk_i = np.concatenate([
            bq_i[x1_idx], bk_i[x1_idx],
            bq_i[x2_idx], bk_i[x2_idx],
            bq_i[rest_idx], bk_i[rest_idx]], axis=0)
        wqk_t = np.ascontiguousarray(Wqk_i.T.astype(ml_dtypes.bfloat16))
        wv_t = np.ascontiguousarray(Wv_i.T.astype(ml_dtypes.bfloat16))
        Wo_i = out_w[rs]
        wo_t = np.ascontiguousarray(Wo_i.T.astype(ml_dtypes.bfloat16))
        cb_i = (out_b[rs] + Wo_i @ bv).astype(np.float32)[None, :]
        bqk_r = np.ascontiguousarray(bqk_i.reshape(5, 128).T)
        in_maps.append({
            "xTc": xTc,
            "wqk_t": wqk_t,
            "wv_t": wv_t,
            "wo_t": wo_t,
            "bqk": bqk_r,
            "cb": cb_i,
            "maskm": maskm,
            "cos8": cos8,
            "sin8": sin8,
            "ones128": ones128,
            "vpad": vpad,
        })
    return in_maps, plan


def get_runner(mask_plan, reps=1):
    key = (mask_plan, reps)
    if key not in _RUNNERS:
        _RUNNERS[key] = Runner(mask_plan, reps=reps)
    return _RUNNERS[key]


def kernel(x, Wqkv_w, Wqkv_b, out_w, out_b, mask):
    in_maps, plan = _host_prep(x, Wqkv_w, Wqkv_b, out_w, out_b, mask)
    runner = get_runner(plan)
    results = runner(in_maps)
    y = np.concatenate([results[i]["y"] for i in range(NCORES)], axis=1)
    return y.reshape(1, L, C)
